# revision 35
# baseline (speedup 1.0000x reference)
"""Trainium2 Bass kernel for nn_AttentionLayer (B=64, F=1024, K=1024).

Reference computation (per batch b):
    scores[k, g] = sum_f input[b, f, k] * weight[f, g] + bias[g]
    alpha        = softmax(scores, axis=g)
    out[b, f, k] = input[b, f, k] * alpha[k, f]

Strategy: data-parallel over batch across 8 NeuronCores (8 batches/core).
Per batch, everything is computed in the transposed [g, k] layout so that no
transposes are ever needed:
    scoresT[g, k] = sum_f W[f, g] * X[f, k]      (lhsT = W chunk, rhs = X chunk)
    E[g, k]  = exp(scoresT + bias[g])            (ScalarE, bias is per-partition)
    T[g, k]  = sum over the 8 g-chunk tiles      (7 VectorE adds, hidden under
                                                  the matmuls)
    S[., k]  = sum_g T[g, k]                     (ONE matmul with ones[128,128]
                                                  stationary -> sum replicated
                                                  across partitions = free bcast)
    D = 1/S  (reciprocal_approx_fast)
    out[f, k] = X[f, k] * E[f, k] * D[k]         (VectorE, g === f axis)

Matmuls run with float32r operands (1 cyc/row vs 4 for fp32; max rel err vs
the fp32 reference ~3.3e-4). The work is software-pipelined over half-batch
"slabs" (k split in two) so PE (~224us), DVE (~229us) and DMA (~216us) run
balanced at >85% occupancy; measured ~258us/core for the 8-batch shard.
"""

import sys
from contextlib import ExitStack

import numpy as np

for _p in ("/opt/trn_rl_repo", "/root/.axon_site/_ro/trn_rl_repo"):
    if _p not in sys.path:
        sys.path.append(_p)

import concourse.bacc as bacc
import concourse.bass as bass
import concourse.mybir as mybir
import concourse.tile as tile
from concourse.bass_utils import run_bass_kernel_spmd

N_CORES = 8
B, F, K = 64, 1024, 1024
BPC = B // N_CORES            # batches per core
P = 128                       # SBUF partitions
NF = F // P                   # f (contraction) chunks
NG = F // P                   # g (feature/output-partition) chunks
KC = 512                      # moving free-dim chunk (fp32 max = 512)
NK = K // KC

FP32 = mybir.dt.float32
F32R = mybir.dt.float32r

EXP = mybir.ActivationFunctionType.Exp


def _build(mm_f32r: bool = True, bpc: int = BPC, reps: int = 1):
    nc = bacc.Bacc("TRN2", target_bir_lowering=False, debug=False)

    mmdt = F32R if mm_f32r else FP32
    x_d = nc.dram_tensor("x", [bpc, F, K], mmdt, kind="ExternalInput").ap()
    w_d = nc.dram_tensor("w", [F, F], mmdt, kind="ExternalInput").ap()
    b_d = nc.dram_tensor("b", [1, F], FP32, kind="ExternalInput").ap()
    ones_d = nc.dram_tensor("ones", [P, P], mmdt, kind="ExternalInput").ap()
    o_d = nc.dram_tensor("out", [bpc, F, K], FP32, kind="ExternalOutput").ap()

    def as_f32(ap):
        return ap.bitcast(FP32) if mm_f32r else ap

    with tile.TileContext(nc) as tc, ExitStack() as ctx:
        w_pool = ctx.enter_context(tc.tile_pool(name="w", bufs=1))
        c_pool = ctx.enter_context(tc.tile_pool(name="const", bufs=1))
        x_pool = ctx.enter_context(tc.tile_pool(name="x", bufs=16))
        e_pool = ctx.enter_context(tc.tile_pool(name="e", bufs=12))
        p_pool = ctx.enter_context(tc.tile_pool(name="pp", bufs=12))
        t_pool = ctx.enter_context(tc.tile_pool(name="t", bufs=3))
        d_pool = ctx.enter_context(tc.tile_pool(name="d", bufs=3))
        o_pool = ctx.enter_context(tc.tile_pool(name="o", bufs=4))
        sc_psum = ctx.enter_context(tc.tile_pool(name="sc", bufs=6, space="PSUM"))
        s_psum = ctx.enter_context(tc.tile_pool(name="s", bufs=2, space="PSUM"))

        # ---- constants (loaded once; W is interleaved with batch-0 X below)
        bias_sb = c_pool.tile([P, NG], FP32)
        nc.sync.dma_start(
            out=bias_sb[:], in_=b_d.rearrange("o (c p) -> (o p) c", p=P)
        )
        ones_sb = c_pool.tile([P, P], mmdt)
        nc.sync.dma_start(out=ones_sb[:], in_=ones_d)

        # w_sb[p, fc*F + g] = W[fc*128 + p, g]
        w_sb = w_pool.tile([P, NF * F], mmdt)

        def w_tile(fc, gc):
            off = fc * F + gc * P
            return w_sb[:, off : off + P]

        def prefetch_x(b, with_w=False):
            x_tiles = []
            for fc in range(NF):
                x_t = x_pool.tile([P, K], mmdt, tag="x")
                if with_w:
                    # startup: interleave the g-low half of W with batch-0's
                    # kc=0 X halves so slab 0's first matmul groups start as
                    # early as possible; the rest streams in behind them
                    nc.sync.dma_start(
                        out=w_sb[:, fc * F : fc * F + F // 2],
                        in_=w_d[fc * P : (fc + 1) * P, 0 : F // 2],
                    )
                    nc.sync.dma_start(
                        out=x_t[:, 0:KC], in_=x_d[b, fc * P : (fc + 1) * P, 0:KC]
                    )
                else:
                    nc.sync.dma_start(
                        out=x_t[:], in_=x_d[b, fc * P : (fc + 1) * P, :]
                    )
                x_tiles.append(x_t)
            if with_w:
                for fc in range(NF):
                    nc.sync.dma_start(
                        out=w_sb[:, fc * F + F // 2 : (fc + 1) * F],
                        in_=w_d[fc * P : (fc + 1) * P, F // 2 : F],
                    )
                for fc in range(NF):
                    nc.sync.dma_start(
                        out=x_tiles[fc][:, KC:K],
                        in_=x_d[b, fc * P : (fc + 1) * P, KC:K],
                    )
            return x_tiles

        def slab_main(b, kc, x_tiles):
            """Main matmuls + exp + partial E-sum for one (batch, k-half) slab.

            The 8 exp tiles are accumulated with 7 DVE adds (running in the
            shadow of the matmuls) so the PE only does ONE ones-matmul per
            slab for the partition sum instead of 8.
            """
            ks = slice(kc * KC, (kc + 1) * KC)
            e_tiles = []
            p_tiles = []
            t_t = None
            for gc in range(NG):
                sc = sc_psum.tile([P, KC], FP32, tag="sc")
                for fc in range(NF):
                    nc.tensor.matmul(
                        sc[:],
                        lhsT=w_tile(fc, gc),
                        rhs=x_tiles[fc][:, ks],
                        start=(fc == 0),
                        stop=(fc == NF - 1),
                    )
                e_t = e_pool.tile([P, KC], mmdt, tag="e")
                nc.scalar.activation(
                    e_t[:], sc[:], EXP, bias=bias_sb[:, gc : gc + 1], scale=1.0
                )
                e_tiles.append(e_t)
                # accumulate E0..E6 on DVE; E7 joins via a second PE
                # ones-matmul in slab_out (keeps PE and DVE balanced)
                if gc == 1:
                    t_t = t_pool.tile([P, KC], mmdt, tag="t")
                    nc.vector.tensor_add(
                        t_t[:], as_f32(e_tiles[0][:]), as_f32(e_t[:])
                    )
                elif 1 < gc < NG - 1:
                    nc.vector.tensor_add(t_t[:], as_f32(t_t[:]), as_f32(e_t[:]))
                # P = X * E needs no denominator -> runs in the shadow of the
                # matmuls, leaving only P * (1/S) for the slab epilogue
                p_t = p_pool.tile([P, KC], FP32, tag="pp")
                nc.vector.tensor_mul(
                    p_t[:], as_f32(x_tiles[gc][:, ks]), as_f32(e_t[:])
                )
                p_tiles.append(p_t)
            return e_tiles, p_tiles, t_t

        def slab_out(b, kc, x_tiles, p_tiles, t_t, e_last):
            """Partition-sum matmuls + reciprocal + final scale + DMA out."""
            ks = slice(kc * KC, (kc + 1) * KC)
            s_t = s_psum.tile([P, KC], FP32, tag="s")
            nc.tensor.matmul(
                s_t[:], lhsT=ones_sb[:], rhs=t_t[:], start=True, stop=False
            )
            nc.tensor.matmul(
                s_t[:], lhsT=ones_sb[:], rhs=e_last[:], start=False, stop=True
            )
            d_t = d_pool.tile([P, KC], FP32, tag="d")
            nc.vector.reciprocal_approx_fast(d_t[:], s_t[:])
            for fc in range(NF):
                o_t = o_pool.tile([P, KC], FP32, tag="o")
                nc.vector.tensor_mul(o_t[:], p_tiles[fc][:], d_t[:])
                nc.sync.dma_start(
                    out=o_d[b, fc * P : (fc + 1) * P, ks], in_=o_t[:]
                )

        # software pipeline over half-batch slabs: the PE stream is
        # [sums s-1][mains s][sums s][mains s+1]... so each slab's DVE chain
        # (recip + muls) overlaps the next slab's matmuls, and the kernel
        # tail after the very last main matmul is only one slab's epilogue.
        prev = None
        first = True
        for _ in range(reps):
            for b in range(bpc):
                x_tiles = prefetch_x(b, with_w=first)
                first = False
                for kc in range(NK):
                    if prev is not None:
                        slab_out(*prev)
                    e_tiles, p_tiles, t_t = slab_main(b, kc, x_tiles)
                    prev = (b, kc, x_tiles, p_tiles, t_t, e_tiles[NG - 1])
        slab_out(*prev)

    nc.compile()
    return nc


_NC = None


def _get_nc():
    global _NC
    if _NC is None:
        _NC = _build()
    return _NC


def kernel(**inputs) -> np.ndarray:
    x = np.ascontiguousarray(np.asarray(inputs["input"], dtype=np.float32))
    w = np.ascontiguousarray(np.asarray(inputs["weight"], dtype=np.float32))
    b = np.ascontiguousarray(np.asarray(inputs["bias"], dtype=np.float32))

    nc = _get_nc()
    ones = np.ones((P, P), dtype=np.float32)
    in_maps = [
        {"x": x[c * BPC : (c + 1) * BPC], "w": w, "b": b, "ones": ones}
        for c in range(N_CORES)
    ]
    res = run_bass_kernel_spmd(nc, in_maps, list(range(N_CORES)))
    return np.concatenate([res.results[c]["out"] for c in range(N_CORES)], axis=0)
